# revision 47
# baseline (speedup 1.0000x reference)
"""Multi-head attention kernel for Trainium2 (8 NeuronCores, data-parallel over batch).

Reference computation (per batch b of 8):
    x:  [1024, 768]  (tokens x channels, n = 32*32)
    qkv = x @ qkv_w.T                    -> [1024, 2304]
    q, k, v per head (12 heads, dh=64)
    S = q @ k.T * dh**-0.5; P = softmax(S); O = P @ v
    out = concat_heads(O) @ proj_w.T + proj_b
Each core processes one batch element independently (no collectives).

On-chip layouts (bf16 compute, fp32 PSUM accumulation):
    x_all   [128c, 6ct, 1024t]   x^T        wq_all  [128c, 6ct, 2304o]  qkv_w^T
    wp_all  [128c, 6ct, 768o]    proj_w^T   qkT[i]  [128o, 1024t]  q^T(0-5)/k^T(6-11)
    Vt[tt]  [128t, 12h, 65]      v + ones column per head
    E(s,h)  [128j, 8jt, 512i]    exp(S^T) for one 512-token i-half
    OT[g]   [128c, 1024t]        attention out^T per head pair

Key structure (HW-measured: each matmul pays an unhidden ~(ldweights rows)
cycle cost, and the Tile framework keeps PER-ENGINE PROGRAM ORDER, so any
consumer stall head-of-line-blocks everything behind it on that engine):

1. Attention is software-pipelined at half-pair granularity: 12 steps
   s=(g,ic) each emit S-score units for step s interleaved with AV matmuls
   of step s-1 and "filler" work (QKV of pair g+1, V chunks, weight
   transposes, first projection pass), so the PE never waits for the
   exp(ACT) stream and the ACT engine is continuously fed.
2. S^T matmuls alternate head quadrants on the PE array (rows 0-63 / 64-127,
   tile_position) - consecutive same-quadrant 64-row matmuls cost ~1.7x more.
3. Loads: 2 row-tiles per DMA instruction (only 8 DMAHW completion-sem lanes
   exist; more DMA instructions = lane-recycle waits that serialize), all on
   the SP HWDGE queue (the ACT queue belongs to the exp stream; an x-load
   burst uses it only during the prologue). f32->bf16 casts on Pool, operand
   transposes on the PE vs a bf16 identity (DMA-xbar transposes were tried
   and are correct but their extra HWDGE instructions serialize the load
   stream through the 8 sem lanes).
4. softmax: no max subtraction (scores are O(1)); denominators via a ones
   column in V; normalize = DVE reciprocal -> Pool partition_broadcast ->
   deferred DVE multiply (deferred so its wait on the broadcast never
   delays the qkT/V PSUM-drain copies queued behind it on the DVE).
5. proj pass 1 (head pairs 0-3 + bias) rides inside the last pipeline steps;
   pass 2 (pairs 4-5) interleaves with the final AV groups; out DMA per
   token tile on the SP queue.
"""

import numpy as np

import concourse.bass as bass
import concourse.mybir as mybir
import concourse.tile as tile
from concourse import bacc
from concourse.masks import make_identity

# Problem constants (hardcoded per contract)
B = 8
N = 1024          # tokens per batch (32*32)
C = 768           # channels
H = 12            # heads
DH = 64           # head dim
O3 = 3 * C        # 2304
SCALE = DH ** -0.5
NCORES = 8

F32 = mybir.dt.float32
BF16 = mybir.dt.bfloat16

CT = C // 128     # 6 c-tiles
TT = N // 128     # 8 token tiles
IC = N // 512     # 2 i-chunks of 512
JT = N // 128     # 8 j-tiles
NSTEP = 12        # (pair g, i-half ic) pipeline steps


def _build_nc(dbg=False, repeat=1, phases="full"):
    nc = bacc.Bacc("TRN2", target_bir_lowering=False, debug=False, num_devices=NCORES)

    x_d = nc.dram_tensor("x", [N, C], F32, kind="ExternalInput").ap()
    qkvw_d = nc.dram_tensor("qkv_w", [O3, C], F32, kind="ExternalInput").ap()
    projw_d = nc.dram_tensor("proj_w", [C, C], F32, kind="ExternalInput").ap()
    projb_d = nc.dram_tensor("proj_b", [C], F32, kind="ExternalInput").ap()
    out_d = nc.dram_tensor("out", [N, C], F32, kind="ExternalOutput").ap()

    with tile.TileContext(nc) as tc:
        _emit(nc, tc, x_d, qkvw_d, projw_d, projb_d, out_d, dbg=dbg, repeat=repeat,
              phases=phases)
    nc.compile()
    return nc


def _emit(nc, tc, x_d, qkvw_d, projw_d, projb_d, out_d, dbg=False, repeat=1,
          phases="full"):
    from collections import deque
    from contextlib import ExitStack

    # phases: "load" < "qkv" < "attn" < "full" — emit only a prefix of the
    # compute pipeline (HW phase-bisection benching; grader always uses full).
    LV = {"load": 0, "qkv": 1, "attn": 2, "full": 3}[phases]

    with ExitStack() as ctx:
        sb = lambda name, bufs: ctx.enter_context(tc.tile_pool(name=name, bufs=bufs))
        ps = lambda name, bufs: ctx.enter_context(
            tc.tile_pool(name=name, bufs=bufs, space="PSUM")
        )

        stage_pool = sb("stage", 4)      # fp32 load staging [128, 2, 768]
        bfst_pool = sb("bfst", 4)        # bf16 cast staging [128, 2, 768]
        big_pool = sb("big", 1)          # x_all / wq_all / wp_all
        qkT_pool = sb("qkT", 12)
        v_pool = sb("vbf", TT)
        e_pool = sb("ebf", 4)            # 4 live: 2 writing (step s) + 2 read (s-1)
        ot_sb_pool = sb("otsb", CT)
        pjp_pool = sb("pjpart", TT)
        rec_pool = sb("rec", 4)
        bias_pool = sb("bias", 1)
        out_pool = sb("outsb", 3)

        qkv_ps = ps("qkvps", 2)          # 1 bank each: QKV / V / proj groups
        sps_ps = ps("sps", 2)            # 2 banks each: S^T units
        ot_ps = ps("otps", 2)            # 1 bank each: O^T + colsum

        # ---------------- persistent tiles ----------------
        ident = big_pool.tile([128, 128], BF16, tag="ident")
        make_identity(nc, ident)

        x_all = big_pool.tile([128, CT, N], BF16, tag="x_all")
        wq_all = big_pool.tile([128, CT, O3], BF16, tag="wq_all")
        wp_all = big_pool.tile([128, CT, C], BF16, tag="wp_all")

        qkT = [
            qkT_pool.tile([128, N], BF16, tag="qkT", name=f"qkT_{i}") for i in range(12)
        ]
        Vt = [
            v_pool.tile([128, H, DH + 1], BF16, tag="vbf", name=f"V_{i}")
            for i in range(TT)
        ]
        OT = [
            ot_sb_pool.tile([128, N], BF16, tag="otsb", name=f"OT_{i}")
            for i in range(CT)
        ]
        pj_partial = [
            pjp_pool.tile([128, C], BF16, tag="pjpart", name=f"pjp_{i}")
            for i in range(TT)
        ]

        # bias broadcast to all partitions (fp32)
        bias_row = bias_pool.tile([1, C], F32, tag="biasrow")
        nc.gpsimd.dma_start(out=bias_row, in_=projb_d[None, :])
        bias_bc = bias_pool.tile([128, C], F32, tag="biasbc")
        nc.gpsimd.partition_broadcast(bias_bc, bias_row)

        # ------------- load + Pool-cast, then PE-transpose as a filler -------------
        # src row-tile rt of a [rows, 768] fp32 matrix -> dst_all[:, :, rt*128+...]
        # ALL loads ride the SP HWDGE queue only (the ACT queue belongs to the
        # exp stream; only 8 DMAHW sem lanes exist and sharing them across
        # queues with cross-engine waits serializes everything). The load
        # stream self-paces: ld(i+3) waits cast(i) (stage WAR) and cast(i+4)
        # waits transpose(i) (bfst WAR), so SBUF staging stays bounded.
        bfst_of = {}

        # Load TWO consecutive row-tiles per DMA instruction (halves the DMA
        # instruction + semaphore count: the HW load chain runs ~3us per DMA
        # instruction regardless of size). DRAM rows rt0*128 .. rt0*128+255
        # land as stage[:, j, :] = rows (rt0+j)*128 + p.
        def load_rt2(src, rt0, tagc, dma_eng=None, cast_eng=None):
            stg = stage_pool.tile([128, 2, C], F32, tag="stage", name="stg")
            src2 = src[rt0 * 128:(rt0 + 2) * 128, :]
            (dma_eng or nc.sync).dma_start(
                out=stg, in_=src2.rearrange("(b p) c -> p b c", p=128)
            )
            stgb = bfst_pool.tile([128, 2, C], BF16, tag="bfst", name="stgb")
            (cast_eng or nc.gpsimd).tensor_copy(stgb, stg)
            bfst_of[(tagc, rt0)] = stgb[:, 0, :]
            bfst_of[(tagc, rt0 + 1)] = stgb[:, 1, :]

        # DMA-xbar transpose of one loaded w batch into the ^T layout (zero
        # PE cycles). Rides the SP queue lagged one batch behind its load so
        # the SEQ-held wait on the Pool cast is satisfied on arrival and the
        # 8 DMAHW sem lanes keep recycling promptly.
        def wdmat(tagc, r0, dst_all):
            for j in (0, 1):
                r = r0 + j
                nc.sync.dma_start_transpose(
                    dst_all[:, :, r * 128:(r + 1) * 128],
                    bfst_of.pop((tagc, r)),
                )

        # 6 PE transposes (vs bf16 identity) + DVE copy into the ^T layout
        def tp_fill(key, dst_all, rt):
            def go():
                stgb = bfst_of.pop(key)
                tp = qkv_ps.tile([128, CT, 128], BF16, tag="qkvps", name="tp")
                for ct in range(CT):
                    nc.tensor.transpose(
                        tp[:, ct, :], stgb[:, ct * 128:(ct + 1) * 128], ident
                    )
                nc.vector.tensor_copy(dst_all[:, :, rt * 128:(rt + 1) * 128], tp)
            return go

        # ---------------- compute building blocks ----------------
        # QK(g): q rows o in [g*128,+128) -> qkT[g], k rows -> qkT[6+g].
        # Emitted as 8 half-group closures (3 matmuls each; 2nd half + copy).
        def qk_halves(g):
            halves = []
            for oi, obase in enumerate((g * 128, C + g * 128)):
                dst = qkT[g] if oi == 0 else qkT[6 + g]
                for ic in range(IC):
                    st = {}
                    def h1(st=st, obase=obase, ic=ic):
                        acc = qkv_ps.tile([128, 512], F32, tag="qkvps", name="qk_acc")
                        st["acc"] = acc
                        for ct in range(3):
                            nc.tensor.matmul(
                                acc,
                                lhsT=wq_all[:, ct, obase:obase + 128],
                                rhs=x_all[:, ct, ic * 512:(ic + 1) * 512],
                                start=(ct == 0),
                                stop=False,
                            )
                    def h2(st=st, obase=obase, ic=ic, dst=dst):
                        acc = st["acc"]
                        for ct in range(3, CT):
                            nc.tensor.matmul(
                                acc,
                                lhsT=wq_all[:, ct, obase:obase + 128],
                                rhs=x_all[:, ct, ic * 512:(ic + 1) * 512],
                                start=False,
                                stop=(ct == CT - 1),
                            )
                        nc.vector.tensor_copy(dst[:, ic * 512:(ic + 1) * 512], acc)
                    halves += [h1, h2]
            return halves

        # V group: v rows o in [1536 + oc*384,+384) for token tile tt
        def v_group(oc, tt):
            def go():
                acc = qkv_ps.tile([128, 384], F32, tag="qkvps", name="v_acc")
                for ct in range(CT):
                    nc.tensor.matmul(
                        acc,
                        lhsT=x_all[:, ct, tt * 128:(tt + 1) * 128],
                        rhs=wq_all[:, ct, 2 * C + oc * 384:2 * C + (oc + 1) * 384],
                        start=(ct == 0),
                        stop=(ct == CT - 1),
                    )
                if oc == 0:
                    nc.vector.memset(Vt[tt][:, :, DH:DH + 1], 1.0)
                nc.vector.tensor_copy(
                    Vt[tt][:, 6 * oc:6 * (oc + 1), 0:DH],
                    acc.rearrange("p (h d) -> p h d", d=DH),
                )
            return go

        # proj pass 1: head pairs 0-3, bias folded in, bf16 partial
        def pj1_group(tt, oc):
            def go():
                acc = qkv_ps.tile([128, 384], F32, tag="qkvps", name="pj_acc")
                for g in range(4):
                    nc.tensor.matmul(
                        acc,
                        lhsT=OT[g][:, tt * 128:(tt + 1) * 128],
                        rhs=wp_all[:, g, oc * 384:(oc + 1) * 384],
                        start=(g == 0),
                        stop=(g == 3),
                    )
                nc.vector.tensor_add(
                    pj_partial[tt][:, oc * 384:(oc + 1) * 384],
                    acc,
                    bias_bc[:, oc * 384:(oc + 1) * 384],
                )
            return go

        # proj pass 2a: fold the head-pair-4 contribution into pj_partial
        # in place (rides inside steps 10-11 once OT[4] halves are ready;
        # one extra bf16 rounding per element, well inside the error budget)
        def pj2a_group(tt, oc):
            def go():
                acc = qkv_ps.tile([128, 384], F32, tag="qkvps", name="pj_acc")
                nc.tensor.matmul(
                    acc,
                    lhsT=OT[4][:, tt * 128:(tt + 1) * 128],
                    rhs=wp_all[:, 4, oc * 384:(oc + 1) * 384],
                    start=True,
                    stop=True,
                )
                nc.vector.tensor_add(
                    pj_partial[tt][:, oc * 384:(oc + 1) * 384],
                    acc,
                    pj_partial[tt][:, oc * 384:(oc + 1) * 384],
                )
            return go

        # proj pass 2 (epilogue): head pairs 4-5 + partial, out DMA per token
        # tile, alternating HWDGE queues to halve the serial output tail
        def pj2_tile(tt):
            osb = out_pool.tile([128, C], F32, tag="outsb", name="osb")
            for oc in range(2):
                acc = qkv_ps.tile([128, 384], F32, tag="qkvps", name="pj_acc")
                for g in (4, 5):
                    nc.tensor.matmul(
                        acc,
                        lhsT=OT[g][:, tt * 128:(tt + 1) * 128],
                        rhs=wp_all[:, g, oc * 384:(oc + 1) * 384],
                        start=(g == 4),
                        stop=(g == 5),
                    )
                nc.vector.tensor_add(
                    osb[:, oc * 384:(oc + 1) * 384],
                    acc,
                    pj_partial[tt][:, oc * 384:(oc + 1) * 384],
                )
            eng = nc.scalar if tt % 2 else nc.sync
            eng.dma_start(out=out_d[tt * 128:(tt + 1) * 128, :], in_=osb)

        # ---------------- pipelined emission per repeat ----------------
        for _rep in range(repeat):
            filler_q = deque()

            def drain(n):
                for _ in range(n):
                    if not filler_q:
                        return
                    filler_q.popleft()()

            E_of_step = {}

            # S unit: scores for j-tiles 2u,2u+1 of both heads of pair g,
            # i-columns [ic*512,+512); exp -> E. Heads alternate PE quadrants.
            def s_unit(s, u):
                g, ic = divmod(s, 2)
                h0, h1 = 2 * g, 2 * g + 1
                Ecur = E_of_step[s]
                un = {
                    h: sps_ps.tile([128, 2, 512], F32, tag="sps", name="sT")
                    for h in (h0, h1)
                }
                for q in range(2):
                    jt = 2 * u + q
                    for h in (h0, h1):
                        hoff = (h % 2) * DH
                        nc.tensor.matmul(
                            un[h][:, q, :],
                            lhsT=qkT[6 + g][hoff:hoff + DH, jt * 128:(jt + 1) * 128],
                            rhs=qkT[g][hoff:hoff + DH, ic * 512:(ic + 1) * 512],
                            start=True,
                            stop=True,
                        )
                for h in (h0, h1):
                    nc.scalar.activation(
                        Ecur[h][:, 2 * u:2 * u + 2, :],
                        un[h],
                        mybir.ActivationFunctionType.Exp,
                        scale=SCALE,
                    )

            # AV for step s_av: per head, [O^T | colsum] = [V|1]^T @ E^T over
            # 8 j-tiles (2 half-closures), then normalize via reciprocal +
            # Pool partition_broadcast + DVE multiply into OT.
            # deferred normalize multiplies: emitted only after the next
            # filler drain, so their DVE-queue wait on the Pool broadcast
            # never delays the qkT/V copies that recycle qkv_ps for the PE.
            pending_mul = []

            def flush_muls():
                while pending_mul:
                    pending_mul.pop(0)()

            def av_halves(s_av):
                g, ic = divmod(s_av, 2)
                Eprev = E_of_step.pop(s_av)
                halves = []
                for h in (2 * g, 2 * g + 1):
                    st = {}
                    def a1(h=h, st=st):
                        acc = ot_ps.tile([65, 512], F32, tag="otps", name="ot_acc")
                        st["acc"] = acc
                        for jt in range(4):
                            nc.tensor.matmul(
                                acc,
                                lhsT=Vt[jt][:, h, :],
                                rhs=Eprev[h][:, jt, :],
                                start=(jt == 0),
                                stop=False,
                            )
                    def a2(h=h, st=st, g=g, ic=ic):
                        acc = st["acc"]
                        for jt in range(4, JT):
                            nc.tensor.matmul(
                                acc,
                                lhsT=Vt[jt][:, h, :],
                                rhs=Eprev[h][:, jt, :],
                                start=False,
                                stop=(jt == JT - 1),
                            )
                        rec = rec_pool.tile([1, 512], F32, tag="rec", name="rec")
                        nc.vector.reciprocal(rec, acc[64:65, :])
                        rbc = rec_pool.tile([64, 512], F32, tag="rbc", name="rbc")
                        nc.gpsimd.partition_broadcast(rbc, rec)
                        def mul(acc=acc, rbc=rbc, h=h, g=g, ic=ic):
                            hoff = (h % 2) * DH
                            nc.vector.tensor_mul(
                                OT[g][hoff:hoff + 64, ic * 512:(ic + 1) * 512],
                                acc[0:64, :],
                                rbc,
                            )
                        pending_mul.append(mul)
                    halves += [a1, a2]
                return halves

            # ---- load stream in consumption order, 2 row-tiles per DMA.
            # x rides BOTH HWDGE queues with casts split Pool/DVE (prologue:
            # exp stream not yet running); w/wp ride the SP queue + Pool.
            for bi, rt0 in enumerate((0, 2, 4, 6)):
                load_rt2(x_d, rt0, "x",
                         dma_eng=nc.scalar if bi % 2 else nc.sync,
                         cast_eng=nc.vector if bi % 2 else nc.gpsimd)
            for r0 in (0, 6, 12, 14, 16, 2, 8, 4, 10):
                load_rt2(qkvw_d, r0, "w")
            for r0 in (0, 2, 4):
                load_rt2(projw_d, r0, "p")

            def Tw(r):
                return tp_fill(("w", r), wq_all, r)

            # prologue compute: QK(0) i-chunk 0 only needs x row-tiles 0-3
            for rt in range(4):
                tp_fill(("x", rt), x_all, rt)()
            Tw(0)(); Tw(6)()
            qh0 = qk_halves(0) if LV >= 1 else None
            if qh0:
                qh0[0](); qh0[1](); qh0[4](); qh0[5]()   # q/k i-chunk 0
            for rt in range(4, TT):
                tp_fill(("x", rt), x_all, rt)()
            if qh0:
                qh0[2](); qh0[3](); qh0[6](); qh0[7]()   # q/k i-chunk 1

            if LV >= 2:
                pending_qk = []
                for s in range(NSTEP):
                    g, ic = divmod(s, 2)
                    # push fillers whose deps are satisfied by now. V(oc)
                    # groups (+ their weight transposes) go BEFORE the next
                    # pair's QK so they are emitted (drained) before the
                    # first AV reader of their Vt tiles. QK(g+1) is split
                    # q-halves at step 2g / k-halves at step 2g+1 (and V(1)
                    # across s2/s3) so no step runs dry of filler work — a
                    # dry step leaves the PE gated on the exp stream with no
                    # slack. CAUTION: every filler must DRAIN before the
                    # first emitted consumer of what it writes (drain budget
                    # below is sized so QK(g+1) is fully emitted before the
                    # S(g+1) units of step 2g+2 — a backlog here is a race,
                    # not a slowdown).
                    if s in (0, 2):
                        oc = s // 2
                        filler_q.extend(Tw(r) for r in range(12 + 3 * oc,
                                                            15 + 3 * oc))
                        filler_q.extend(
                            v_group(oc, tt)
                            for tt in (range(TT) if oc == 0 else range(4))
                        )
                    if s == 3:
                        filler_q.extend(v_group(1, tt) for tt in range(4, TT))
                    if g < 5 and LV >= 1:
                        if ic == 0:
                            filler_q.append(Tw(g + 1))
                            filler_q.append(Tw(7 + g))
                            qh = qk_halves(g + 1)
                            filler_q.extend(qh[:4])      # q-halves
                            pending_qk = qh[4:]          # k-halves
                        else:
                            filler_q.extend(pending_qk)
                            pending_qk = []
                    if LV >= 3 and s == 8:
                        filler_q.extend(
                            tp_fill(("p", r), wp_all, r) for r in range(CT)
                        )
                    if LV >= 3 and s >= 8:
                        tts = (s - 8) * 2
                        filler_q.extend(
                            pj1_group(tt, oc)
                            for tt in (tts, tts + 1) for oc in range(2)
                        )

                    E_of_step[s] = {
                        h: e_pool.tile([128, JT, 512], BF16, tag="ebf", name=f"E_{h}")
                        for h in (2 * g, 2 * g + 1)
                    }
                    av = av_halves(s - 1) if s >= 1 else None
                    for u in range(4):
                        s_unit(s, u)
                        if av is not None:
                            av[u]()
                        drain(3 if (s < 4 or s >= 8) else 2)
                        flush_muls()

                # epilogue: AV(5, ic1) interleaved with proj pass 2
                last = av_halves(NSTEP - 1)
                last[0](); last[1]()
                drain(len(filler_q))
                flush_muls()
                if LV >= 3:
                    pj2_tile(0); pj2_tile(1)
                last[2](); last[3]()
                flush_muls()
                if LV >= 3:
                    pj2_tile(2); pj2_tile(3)
                    for tt in range(4, TT):
                        pj2_tile(tt)
            else:
                # bisection stubs: just drain queued work
                if LV >= 1:
                    for g in range(1, 6):
                        Tw(g)(); Tw(6 + g)()
                        for f in qk_halves(g):
                            f()
                    for r in range(12, 18):
                        Tw(r)()
                    for oc in range(2):
                        for tt in range(TT):
                            v_group(oc, tt)()
                    for r in range(CT):
                        tp_fill(("p", r), wp_all, r)()
                bfst_of.clear()

        if dbg:
            taps = {
                "dbg_xall": x_all,
                "dbg_wqall": wq_all,
                "dbg_wpall": wp_all,
                "dbg_qkT0": qkT[0],
                "dbg_qkT6": qkT[6],
                "dbg_V0": Vt[0],
                "dbg_OT0": OT[0],
                "dbg_bias": bias_bc,
            }
            for name, t in taps.items():
                d = nc.dram_tensor(name, list(t.shape), t.dtype, kind="ExternalOutput").ap()
                nc.gpsimd.dma_start(out=d, in_=t)


_NC_CACHE = None


def _get_nc():
    global _NC_CACHE
    if _NC_CACHE is None:
        _NC_CACHE = _build_nc()
    return _NC_CACHE


def kernel(x, qkv_w, proj_w, proj_b, _trace=False):
    from concourse.bass_utils import run_bass_kernel_spmd

    x = np.ascontiguousarray(np.asarray(x, dtype=np.float32))
    qkv_w = np.ascontiguousarray(np.asarray(qkv_w, dtype=np.float32))
    proj_w = np.ascontiguousarray(np.asarray(proj_w, dtype=np.float32))
    proj_b = np.ascontiguousarray(np.asarray(proj_b, dtype=np.float32))

    b, hh, ww, c = x.shape
    assert (b, hh, ww, c) == (B, 32, 32, C)
    xf = x.reshape(B, N, C)

    nc = _get_nc()
    in_maps = [
        {"x": xf[i], "qkv_w": qkv_w, "proj_w": proj_w, "proj_b": proj_b}
        for i in range(NCORES)
    ]
    res = run_bass_kernel_spmd(nc, in_maps, core_ids=list(range(NCORES)), trace=_trace)
    out = np.stack([r["out"] for r in res.results], axis=0).reshape(B, 32, 32, C)
    if _trace:
        kernel._last_results = res
    return out


# revision 48
# speedup vs baseline: 1.0209x; 1.0209x over previous
"""Multi-head attention kernel for Trainium2 (8 NeuronCores, data-parallel over batch).

Reference computation (per batch b of 8):
    x:  [1024, 768]  (tokens x channels, n = 32*32)
    qkv = x @ qkv_w.T                    -> [1024, 2304]
    q, k, v per head (12 heads, dh=64)
    S = q @ k.T * dh**-0.5; P = softmax(S); O = P @ v
    out = concat_heads(O) @ proj_w.T + proj_b
Each core processes one batch element independently (no collectives).

On-chip layouts (bf16 compute, fp32 PSUM accumulation):
    x_all   [128c, 6ct, 1024t]   x^T        wq_all  [128c, 6ct, 2304o]  qkv_w^T
    wp_all  [128c, 6ct, 768o]    proj_w^T   qkT[i]  [128o, 1024t]  q^T(0-5)/k^T(6-11)
    Vt[tt]  [128t, 12h, 65]      v + ones column per head
    E(s,h)  [128j, 8jt, 512i]    exp(S^T) for one 512-token i-half
    OT[g]   [128c, 1024t]        attention out^T per head pair

Key structure (HW-measured: each matmul pays an unhidden ~(ldweights rows)
cycle cost, and the Tile framework keeps PER-ENGINE PROGRAM ORDER, so any
consumer stall head-of-line-blocks everything behind it on that engine):

1. Attention is software-pipelined at half-pair granularity: 12 steps
   s=(g,ic) each emit S-score units for step s interleaved with AV matmuls
   of step s-1 and "filler" work (QKV of pair g+1, V chunks, weight
   transposes, first projection pass), so the PE never waits for the
   exp(ACT) stream and the ACT engine is continuously fed.
2. S^T matmuls alternate head quadrants on the PE array (rows 0-63 / 64-127,
   tile_position) - consecutive same-quadrant 64-row matmuls cost ~1.7x more.
3. Loads: 2 row-tiles per DMA instruction (only 8 DMAHW completion-sem lanes
   exist; more DMA instructions = lane-recycle waits that serialize), all on
   the SP HWDGE queue (the ACT queue belongs to the exp stream; an x-load
   burst uses it only during the prologue). f32->bf16 casts on Pool, operand
   transposes on the PE vs a bf16 identity (DMA-xbar transposes were tried
   and are correct but their extra HWDGE instructions serialize the load
   stream through the 8 sem lanes).
4. softmax: no max subtraction (scores are O(1)); denominators via a ones
   column in V; normalize = DVE reciprocal -> Pool partition_broadcast ->
   deferred DVE multiply (deferred so its wait on the broadcast never
   delays the qkT/V PSUM-drain copies queued behind it on the DVE).
5. proj pass 1 (head pairs 0-3 + bias) rides inside the last pipeline steps;
   pass 2 (pairs 4-5) interleaves with the final AV groups; out DMA per
   token tile on the SP queue.
"""

import numpy as np

import concourse.bass as bass
import concourse.mybir as mybir
import concourse.tile as tile
from concourse import bacc
from concourse.masks import make_identity

# Problem constants (hardcoded per contract)
B = 8
N = 1024          # tokens per batch (32*32)
C = 768           # channels
H = 12            # heads
DH = 64           # head dim
O3 = 3 * C        # 2304
SCALE = DH ** -0.5
NCORES = 8

F32 = mybir.dt.float32
BF16 = mybir.dt.bfloat16

CT = C // 128     # 6 c-tiles
TT = N // 128     # 8 token tiles
IC = N // 512     # 2 i-chunks of 512
JT = N // 128     # 8 j-tiles
NSTEP = 12        # (pair g, i-half ic) pipeline steps


def _build_nc(dbg=False, repeat=1, phases="full"):
    nc = bacc.Bacc("TRN2", target_bir_lowering=False, debug=False, num_devices=NCORES)

    x_d = nc.dram_tensor("x", [N, C], F32, kind="ExternalInput").ap()
    qkvw_d = nc.dram_tensor("qkv_w", [O3, C], F32, kind="ExternalInput").ap()
    projw_d = nc.dram_tensor("proj_w", [C, C], F32, kind="ExternalInput").ap()
    projb_d = nc.dram_tensor("proj_b", [C], F32, kind="ExternalInput").ap()
    out_d = nc.dram_tensor("out", [N, C], F32, kind="ExternalOutput").ap()

    with tile.TileContext(nc) as tc:
        _emit(nc, tc, x_d, qkvw_d, projw_d, projb_d, out_d, dbg=dbg, repeat=repeat,
              phases=phases)
    nc.compile()
    return nc


def _emit(nc, tc, x_d, qkvw_d, projw_d, projb_d, out_d, dbg=False, repeat=1,
          phases="full"):
    from collections import deque
    from contextlib import ExitStack

    # phases: "load" < "qkv" < "attn" < "full" — emit only a prefix of the
    # compute pipeline (HW phase-bisection benching; grader always uses full).
    LV = {"load": 0, "qkv": 1, "attn": 2, "full": 3}[phases]

    with ExitStack() as ctx:
        sb = lambda name, bufs: ctx.enter_context(tc.tile_pool(name=name, bufs=bufs))
        ps = lambda name, bufs: ctx.enter_context(
            tc.tile_pool(name=name, bufs=bufs, space="PSUM")
        )

        stage_pool = sb("stage", 4)      # fp32 load staging [128, 2, 768]
        bfst_pool = sb("bfst", 4)        # bf16 cast staging [128, 2, 768]
        big_pool = sb("big", 1)          # x_all / wq_all / wp_all
        qkT_pool = sb("qkT", 12)
        v_pool = sb("vbf", TT)
        e_pool = sb("ebf", 4)            # 4 live: 2 writing (step s) + 2 read (s-1)
        ot_sb_pool = sb("otsb", CT)
        pjp_pool = sb("pjpart", TT)
        rec_pool = sb("rec", 4)
        bias_pool = sb("bias", 1)
        out_pool = sb("outsb", 3)

        qkv_ps = ps("qkvps", 2)          # 1 bank each: QKV / V / proj groups
        sps_ps = ps("sps", 2)            # 2 banks each: S^T units
        ot_ps = ps("otps", 2)            # 1 bank each: O^T + colsum

        # ---------------- persistent tiles ----------------
        ident = big_pool.tile([128, 128], BF16, tag="ident")
        make_identity(nc, ident)

        x_all = big_pool.tile([128, CT, N], BF16, tag="x_all")
        wq_all = big_pool.tile([128, CT, O3], BF16, tag="wq_all")
        wp_all = big_pool.tile([128, CT, C], BF16, tag="wp_all")

        qkT = [
            qkT_pool.tile([128, N], BF16, tag="qkT", name=f"qkT_{i}") for i in range(12)
        ]
        Vt = [
            v_pool.tile([128, H, DH + 1], BF16, tag="vbf", name=f"V_{i}")
            for i in range(TT)
        ]
        OT = [
            ot_sb_pool.tile([128, N], BF16, tag="otsb", name=f"OT_{i}")
            for i in range(CT)
        ]
        pj_partial = [
            pjp_pool.tile([128, C], BF16, tag="pjpart", name=f"pjp_{i}")
            for i in range(TT)
        ]

        # bias broadcast to all partitions (fp32)
        bias_row = bias_pool.tile([1, C], F32, tag="biasrow")
        nc.gpsimd.dma_start(out=bias_row, in_=projb_d[None, :])
        bias_bc = bias_pool.tile([128, C], F32, tag="biasbc")
        nc.gpsimd.partition_broadcast(bias_bc, bias_row)

        # ------------- load + Pool-cast, then PE-transpose as a filler -------------
        # src row-tile rt of a [rows, 768] fp32 matrix -> dst_all[:, :, rt*128+...]
        # ALL loads ride the SP HWDGE queue only (the ACT queue belongs to the
        # exp stream; only 8 DMAHW sem lanes exist and sharing them across
        # queues with cross-engine waits serializes everything). The load
        # stream self-paces: ld(i+3) waits cast(i) (stage WAR) and cast(i+4)
        # waits transpose(i) (bfst WAR), so SBUF staging stays bounded.
        bfst_of = {}

        # Load TWO consecutive row-tiles per DMA instruction (halves the DMA
        # instruction + semaphore count: the HW load chain runs ~3us per DMA
        # instruction regardless of size). DRAM rows rt0*128 .. rt0*128+255
        # land as stage[:, j, :] = rows (rt0+j)*128 + p.
        def load_rt2(src, rt0, tagc, dma_eng=None, cast_eng=None):
            stg = stage_pool.tile([128, 2, C], F32, tag="stage", name="stg")
            src2 = src[rt0 * 128:(rt0 + 2) * 128, :]
            (dma_eng or nc.sync).dma_start(
                out=stg, in_=src2.rearrange("(b p) c -> p b c", p=128)
            )
            stgb = bfst_pool.tile([128, 2, C], BF16, tag="bfst", name="stgb")
            (cast_eng or nc.gpsimd).tensor_copy(stgb, stg)
            bfst_of[(tagc, rt0)] = stgb[:, 0, :]
            bfst_of[(tagc, rt0 + 1)] = stgb[:, 1, :]

        # DMA-xbar transpose of one loaded w batch into the ^T layout (zero
        # PE cycles). Rides the SP queue lagged one batch behind its load so
        # the SEQ-held wait on the Pool cast is satisfied on arrival and the
        # 8 DMAHW sem lanes keep recycling promptly.
        def wdmat(tagc, r0, dst_all):
            for j in (0, 1):
                r = r0 + j
                nc.sync.dma_start_transpose(
                    dst_all[:, :, r * 128:(r + 1) * 128],
                    bfst_of.pop((tagc, r)),
                )

        # 6 PE transposes (vs bf16 identity) + DVE copy into the ^T layout
        def tp_fill(key, dst_all, rt):
            def go():
                stgb = bfst_of.pop(key)
                tp = qkv_ps.tile([128, CT, 128], BF16, tag="qkvps", name="tp")
                for ct in range(CT):
                    nc.tensor.transpose(
                        tp[:, ct, :], stgb[:, ct * 128:(ct + 1) * 128], ident
                    )
                nc.vector.tensor_copy(dst_all[:, :, rt * 128:(rt + 1) * 128], tp)
            return go

        # ---------------- compute building blocks ----------------
        # QK(g): q rows o in [g*128,+128) -> qkT[g], k rows -> qkT[6+g].
        # Emitted as 8 half-group closures (3 matmuls each; 2nd half + copy).
        def qk_halves(g):
            halves = []
            for oi, obase in enumerate((g * 128, C + g * 128)):
                dst = qkT[g] if oi == 0 else qkT[6 + g]
                for ic in range(IC):
                    st = {}
                    def h1(st=st, obase=obase, ic=ic):
                        acc = qkv_ps.tile([128, 512], F32, tag="qkvps", name="qk_acc")
                        st["acc"] = acc
                        for ct in range(3):
                            nc.tensor.matmul(
                                acc,
                                lhsT=wq_all[:, ct, obase:obase + 128],
                                rhs=x_all[:, ct, ic * 512:(ic + 1) * 512],
                                start=(ct == 0),
                                stop=False,
                            )
                    def h2(st=st, obase=obase, ic=ic, dst=dst):
                        acc = st["acc"]
                        for ct in range(3, CT):
                            nc.tensor.matmul(
                                acc,
                                lhsT=wq_all[:, ct, obase:obase + 128],
                                rhs=x_all[:, ct, ic * 512:(ic + 1) * 512],
                                start=False,
                                stop=(ct == CT - 1),
                            )
                        nc.vector.tensor_copy(dst[:, ic * 512:(ic + 1) * 512], acc)
                    halves += [h1, h2]
            return halves

        # V group: v rows o in [1536 + oc*384,+384) for token tile tt
        def v_group(oc, tt):
            def go():
                acc = qkv_ps.tile([128, 384], F32, tag="qkvps", name="v_acc")
                for ct in range(CT):
                    nc.tensor.matmul(
                        acc,
                        lhsT=x_all[:, ct, tt * 128:(tt + 1) * 128],
                        rhs=wq_all[:, ct, 2 * C + oc * 384:2 * C + (oc + 1) * 384],
                        start=(ct == 0),
                        stop=(ct == CT - 1),
                    )
                if oc == 0:
                    nc.gpsimd.memset(Vt[tt][:, :, DH:DH + 1], 1.0)
                # PSUM drain on ACT (Copy shares the exp table set — no
                # table reload) keeps the DVE queue free for the qkT copies
                # that recycle qkv_ps for the PE
                nc.scalar.activation(
                    Vt[tt][:, 6 * oc:6 * (oc + 1), 0:DH],
                    acc.rearrange("p (h d) -> p h d", d=DH),
                    mybir.ActivationFunctionType.Copy,
                )
            return go

        # proj pass 1: head pairs 0-3, bias folded in, bf16 partial
        def pj1_group(tt, oc):
            def go():
                acc = qkv_ps.tile([128, 384], F32, tag="qkvps", name="pj_acc")
                for g in range(4):
                    nc.tensor.matmul(
                        acc,
                        lhsT=OT[g][:, tt * 128:(tt + 1) * 128],
                        rhs=wp_all[:, g, oc * 384:(oc + 1) * 384],
                        start=(g == 0),
                        stop=(g == 3),
                    )
                nc.vector.tensor_add(
                    pj_partial[tt][:, oc * 384:(oc + 1) * 384],
                    acc,
                    bias_bc[:, oc * 384:(oc + 1) * 384],
                )
            return go

        # proj pass 2a: fold the head-pair-4 contribution into pj_partial
        # in place (rides inside steps 10-11 once OT[4] halves are ready;
        # one extra bf16 rounding per element, well inside the error budget)
        def pj2a_group(tt, oc):
            def go():
                acc = qkv_ps.tile([128, 384], F32, tag="qkvps", name="pj_acc")
                nc.tensor.matmul(
                    acc,
                    lhsT=OT[4][:, tt * 128:(tt + 1) * 128],
                    rhs=wp_all[:, 4, oc * 384:(oc + 1) * 384],
                    start=True,
                    stop=True,
                )
                nc.vector.tensor_add(
                    pj_partial[tt][:, oc * 384:(oc + 1) * 384],
                    acc,
                    pj_partial[tt][:, oc * 384:(oc + 1) * 384],
                )
            return go

        # proj pass 2 (epilogue): head pairs 4-5 + partial, out DMA per token
        # tile, alternating HWDGE queues to halve the serial output tail
        def pj2_tile(tt):
            osb = out_pool.tile([128, C], F32, tag="outsb", name="osb")
            for oc in range(2):
                acc = qkv_ps.tile([128, 384], F32, tag="qkvps", name="pj_acc")
                for g in (4, 5):
                    nc.tensor.matmul(
                        acc,
                        lhsT=OT[g][:, tt * 128:(tt + 1) * 128],
                        rhs=wp_all[:, g, oc * 384:(oc + 1) * 384],
                        start=(g == 4),
                        stop=(g == 5),
                    )
                nc.vector.tensor_add(
                    osb[:, oc * 384:(oc + 1) * 384],
                    acc,
                    pj_partial[tt][:, oc * 384:(oc + 1) * 384],
                )
            eng = nc.scalar if tt % 2 else nc.sync
            eng.dma_start(out=out_d[tt * 128:(tt + 1) * 128, :], in_=osb)

        # ---------------- pipelined emission per repeat ----------------
        for _rep in range(repeat):
            filler_q = deque()

            def drain(n):
                for _ in range(n):
                    if not filler_q:
                        return
                    filler_q.popleft()()

            E_of_step = {}

            # S unit: scores for j-tiles 2u,2u+1 of both heads of pair g,
            # i-columns [ic*512,+512); exp -> E. Heads alternate PE quadrants.
            def s_unit(s, u):
                g, ic = divmod(s, 2)
                h0, h1 = 2 * g, 2 * g + 1
                Ecur = E_of_step[s]
                un = {
                    h: sps_ps.tile([128, 2, 512], F32, tag="sps", name="sT")
                    for h in (h0, h1)
                }
                for q in range(2):
                    jt = 2 * u + q
                    for h in (h0, h1):
                        hoff = (h % 2) * DH
                        nc.tensor.matmul(
                            un[h][:, q, :],
                            lhsT=qkT[6 + g][hoff:hoff + DH, jt * 128:(jt + 1) * 128],
                            rhs=qkT[g][hoff:hoff + DH, ic * 512:(ic + 1) * 512],
                            start=True,
                            stop=True,
                        )
                for h in (h0, h1):
                    nc.scalar.activation(
                        Ecur[h][:, 2 * u:2 * u + 2, :],
                        un[h],
                        mybir.ActivationFunctionType.Exp,
                        scale=SCALE,
                    )

            # AV for step s_av: per head, [O^T | colsum] = [V|1]^T @ E^T over
            # 8 j-tiles (2 half-closures), then normalize via reciprocal +
            # Pool partition_broadcast + DVE multiply into OT.
            # deferred normalize multiplies: emitted only after the next
            # filler drain, so their DVE-queue wait on the Pool broadcast
            # never delays the qkT/V copies that recycle qkv_ps for the PE.
            pending_mul = []

            def flush_muls():
                while pending_mul:
                    pending_mul.pop(0)()

            def av_halves(s_av):
                g, ic = divmod(s_av, 2)
                Eprev = E_of_step.pop(s_av)
                halves = []
                for h in (2 * g, 2 * g + 1):
                    st = {}
                    def a1(h=h, st=st):
                        acc = ot_ps.tile([65, 512], F32, tag="otps", name="ot_acc")
                        st["acc"] = acc
                        for jt in range(4):
                            nc.tensor.matmul(
                                acc,
                                lhsT=Vt[jt][:, h, :],
                                rhs=Eprev[h][:, jt, :],
                                start=(jt == 0),
                                stop=False,
                            )
                    def a2(h=h, st=st, g=g, ic=ic):
                        acc = st["acc"]
                        for jt in range(4, JT):
                            nc.tensor.matmul(
                                acc,
                                lhsT=Vt[jt][:, h, :],
                                rhs=Eprev[h][:, jt, :],
                                start=False,
                                stop=(jt == JT - 1),
                            )
                        rec = rec_pool.tile([1, 512], F32, tag="rec", name="rec")
                        nc.vector.reciprocal(rec, acc[64:65, :])
                        rbc = rec_pool.tile([64, 512], F32, tag="rbc", name="rbc")
                        nc.gpsimd.partition_broadcast(rbc, rec)
                        def mul(acc=acc, rbc=rbc, h=h, g=g, ic=ic):
                            hoff = (h % 2) * DH
                            nc.vector.tensor_mul(
                                OT[g][hoff:hoff + 64, ic * 512:(ic + 1) * 512],
                                acc[0:64, :],
                                rbc,
                            )
                        pending_mul.append(mul)
                    halves += [a1, a2]
                return halves

            # ---- load stream in consumption order, 2 row-tiles per DMA.
            # x rides BOTH HWDGE queues with casts split Pool/DVE (prologue:
            # exp stream not yet running); w/wp ride the SP queue + Pool.
            for bi, rt0 in enumerate((0, 2, 4, 6)):
                load_rt2(x_d, rt0, "x",
                         dma_eng=nc.scalar if bi % 2 else nc.sync,
                         cast_eng=nc.vector if bi % 2 else nc.gpsimd)
            for r0 in (0, 6, 12, 14, 16, 2, 8, 4, 10):
                load_rt2(qkvw_d, r0, "w")
            for r0 in (0, 2, 4):
                load_rt2(projw_d, r0, "p")

            def Tw(r):
                return tp_fill(("w", r), wq_all, r)

            # prologue compute: QK(0) i-chunk 0 only needs x row-tiles 0-3
            for rt in range(4):
                tp_fill(("x", rt), x_all, rt)()
            Tw(0)(); Tw(6)()
            qh0 = qk_halves(0) if LV >= 1 else None
            if qh0:
                qh0[0](); qh0[1](); qh0[4](); qh0[5]()   # q/k i-chunk 0
            for rt in range(4, TT):
                tp_fill(("x", rt), x_all, rt)()
            if qh0:
                qh0[2](); qh0[3](); qh0[6](); qh0[7]()   # q/k i-chunk 1

            if LV >= 2:
                pending_qk = []
                for s in range(NSTEP):
                    g, ic = divmod(s, 2)
                    # push fillers whose deps are satisfied by now. V(oc)
                    # groups (+ their weight transposes) go BEFORE the next
                    # pair's QK so they are emitted (drained) before the
                    # first AV reader of their Vt tiles. QK(g+1) is split
                    # q-halves at step 2g / k-halves at step 2g+1 (and V(1)
                    # across s2/s3) so no step runs dry of filler work — a
                    # dry step leaves the PE gated on the exp stream with no
                    # slack. CAUTION: every filler must DRAIN before the
                    # first emitted consumer of what it writes (drain budget
                    # below is sized so QK(g+1) is fully emitted before the
                    # S(g+1) units of step 2g+2 — a backlog here is a race,
                    # not a slowdown).
                    if s in (0, 2):
                        oc = s // 2
                        filler_q.extend(Tw(r) for r in range(12 + 3 * oc,
                                                            15 + 3 * oc))
                        filler_q.extend(
                            v_group(oc, tt)
                            for tt in (range(TT) if oc == 0 else range(4))
                        )
                    if s == 3:
                        filler_q.extend(v_group(1, tt) for tt in range(4, TT))
                    if g < 5 and LV >= 1:
                        if ic == 0:
                            filler_q.append(Tw(g + 1))
                            filler_q.append(Tw(7 + g))
                            qh = qk_halves(g + 1)
                            filler_q.extend(qh[:4])      # q-halves
                            pending_qk = qh[4:]          # k-halves
                        else:
                            filler_q.extend(pending_qk)
                            pending_qk = []
                    if LV >= 3 and s == 8:
                        filler_q.extend(
                            tp_fill(("p", r), wp_all, r) for r in range(CT)
                        )
                    if LV >= 3 and s >= 8:
                        tts = (s - 8) * 2
                        filler_q.extend(
                            pj1_group(tt, oc)
                            for tt in (tts, tts + 1) for oc in range(2)
                        )

                    E_of_step[s] = {
                        h: e_pool.tile([128, JT, 512], BF16, tag="ebf", name=f"E_{h}")
                        for h in (2 * g, 2 * g + 1)
                    }
                    av = av_halves(s - 1) if s >= 1 else None
                    for u in range(4):
                        s_unit(s, u)
                        if av is not None:
                            av[u]()
                        drain(3 if (s < 4 or s >= 8) else 2)
                        flush_muls()

                # epilogue: AV(5, ic1) interleaved with proj pass 2
                last = av_halves(NSTEP - 1)
                last[0](); last[1]()
                drain(len(filler_q))
                flush_muls()
                if LV >= 3:
                    pj2_tile(0); pj2_tile(1)
                last[2](); last[3]()
                flush_muls()
                if LV >= 3:
                    pj2_tile(2); pj2_tile(3)
                    for tt in range(4, TT):
                        pj2_tile(tt)
            else:
                # bisection stubs: just drain queued work
                if LV >= 1:
                    for g in range(1, 6):
                        Tw(g)(); Tw(6 + g)()
                        for f in qk_halves(g):
                            f()
                    for r in range(12, 18):
                        Tw(r)()
                    for oc in range(2):
                        for tt in range(TT):
                            v_group(oc, tt)()
                    for r in range(CT):
                        tp_fill(("p", r), wp_all, r)()
                bfst_of.clear()

        if dbg:
            taps = {
                "dbg_xall": x_all,
                "dbg_wqall": wq_all,
                "dbg_wpall": wp_all,
                "dbg_qkT0": qkT[0],
                "dbg_qkT6": qkT[6],
                "dbg_V0": Vt[0],
                "dbg_OT0": OT[0],
                "dbg_bias": bias_bc,
            }
            for name, t in taps.items():
                d = nc.dram_tensor(name, list(t.shape), t.dtype, kind="ExternalOutput").ap()
                nc.gpsimd.dma_start(out=d, in_=t)


_NC_CACHE = None


def _get_nc():
    global _NC_CACHE
    if _NC_CACHE is None:
        _NC_CACHE = _build_nc()
    return _NC_CACHE


def kernel(x, qkv_w, proj_w, proj_b, _trace=False):
    from concourse.bass_utils import run_bass_kernel_spmd

    x = np.ascontiguousarray(np.asarray(x, dtype=np.float32))
    qkv_w = np.ascontiguousarray(np.asarray(qkv_w, dtype=np.float32))
    proj_w = np.ascontiguousarray(np.asarray(proj_w, dtype=np.float32))
    proj_b = np.ascontiguousarray(np.asarray(proj_b, dtype=np.float32))

    b, hh, ww, c = x.shape
    assert (b, hh, ww, c) == (B, 32, 32, C)
    xf = x.reshape(B, N, C)

    nc = _get_nc()
    in_maps = [
        {"x": xf[i], "qkv_w": qkv_w, "proj_w": proj_w, "proj_b": proj_b}
        for i in range(NCORES)
    ]
    res = run_bass_kernel_spmd(nc, in_maps, core_ids=list(range(NCORES)), trace=_trace)
    out = np.stack([r["out"] for r in res.results], axis=0).reshape(B, 32, 32, C)
    if _trace:
        kernel._last_results = res
    return out
